# revision 61
# baseline (speedup 1.0000x reference)
"""Trainium2 Bass kernel for segment_reduce MLP (nn_HeadSemantic_35983236006251).

Math shortcut: Linear commutes with segment_sum, so
    pooled = segment_sum(x @ W_in + b_in) = segment_sum(x) @ W_in + counts * b_in
and the kernel reduces to memory-bound streaming of x into per-segment sums,
followed by a tiny MLP on [4096, 256].

Design (vs. the one-hot baseline):
  * x is streamed in fp8 (e4m3) instead of fp32 -- 4x less HBM traffic.
    Host-side quantization uses error feedback (sigma-delta) along each
    (segment, column) chain, so the device's exact-fp32 PSUM accumulation sees
    a segment-sum error of ~1 quantization step instead of ~sqrt(n) steps.
    Measured end-to-end rel err ~4e-3 (gate is 2e-2).
  * No per-tile one-hot build at all: segments are sorted by size on the host
    and assigned one-per-partition; x is re-laid-out in DRAM as per-partition
    row streams.  The segment-sum is then a PSUM accumulation with a CONSTANT
    doubled-identity lhsT in fp8 DoubleRow mode (2 tiles of 128 rows per
    matmul).
  * x is DMAed in big per-partition-contiguous slabs (16 KB/partition),
    round-robined over THREE DGE queues (SP / Activation / Pool) so the
    descriptor rings never throttle the HBM stream; DVE's queue carries the
    small constants and drains results.
  * The MLP runs per 64-segment chunk as soon as its window's pooled sums
    flush, overlapped with streaming of later windows; weights/activations in
    bf16 (PSUM accumulation fp32); biases fused into the DVE PSUM->SBUF
    copies (per-partition tensor_scalar add / add+relu), the counts*b_in term
    via scalar_tensor_tensor against a host-broadcast counts plane.  PSUM
    scratch alternates by chunk parity so chunk q+1's matmuls overlap chunk
    q's drain copies; only the final chunk's drain is tail latency.

Sharding: segments sorted by size desc; window w (of 32) = segments
[128w, 128w+128); slot-group s = windows [8s, 8s+8); core c takes window
8s + c of each group.  All cores share one SPMD program whose per-slot tile
counts are the group maxima.
"""

import sys
import numpy as np
import ml_dtypes
from contextlib import ExitStack

sys.path.insert(0, "/opt/trn_rl_repo")

import concourse.bass as bass
from concourse import mybir
from concourse.bass_utils import run_bass_kernel_spmd

N = 1_000_000
D = 256
NSEG = 4096
N_CORES = 8
NSLOT = 4                  # windows per core
SEG = NSLOT * 128          # segments per core
NCHUNK = 2 * NSLOT         # 64-segment MLP chunks
F32 = mybir.dt.float32
BF16 = mybir.dt.bfloat16
F8 = mybir.dt.float8e4
NPF8 = ml_dtypes.float8_e4m3
NPBF = ml_dtypes.bfloat16
SLAB_PAIRS = 32            # row-tile pairs per DMA slab (64 tiles, 16KB/part)
NQ = 3                     # x DMA queues (SP, Act, Pool)
QSLOTS = [[0, 1, 2], [3, 4, 5], [6, 7, 8]]        # queue-exclusive xbuf slots
QCAP = [99, 99, 99]
NRING = 9
DR = mybir.MatmulPerfMode.DoubleRow
ADD = mybir.AluOpType.add
MAX = mybir.AluOpType.max
MULT = mybir.AluOpType.mult
NCONST = 12                # MLP const DMAs on s_c


def _slab_plan(TP):
    """TP = tiles per slot (even).  Each slab is
    (slot, dram_tile0, npairs, first_of_slot)."""
    slabs = []
    cumslabs = []
    base = 0
    for s, tp in enumerate(TP):
        pairs = tp // 2
        k = 0
        while k < pairs:
            np_ = min(SLAB_PAIRS, pairs - k)
            slabs.append((s, base + 2 * k, np_, k == 0))
            k += np_
        cumslabs.append(len(slabs))
        base += tp
    return slabs, cumslabs


def build_program(TP, NOV):
    nc = bass.Bass()
    TOT = sum(TP)
    OVT = 2 * sum(NOV)
    PAIRS = [tp // 2 for tp in TP]
    slabs, cumslabs = _slab_plan(TP)
    NSLAB = len(slabs)

    xdr_in = nc.declare_dram_parameter("xdr", [128, TOT, D], F8, False)
    xov_in = nc.declare_dram_parameter("xov", [128, OVT, D], F8, False)
    sov_in = nc.declare_dram_parameter("sov", [128, OVT, 128], F8, False)
    id2_in = nc.declare_dram_parameter("id2", [128, 2, 128], F8, False)
    idf_in = nc.declare_dram_parameter("idf", [128, 128], F32, False)
    win_in = nc.declare_dram_parameter("win", [D, D], BF16, False)
    w1_in = nc.declare_dram_parameter("w1", [D, 2 * D], BF16, False)
    w2_in = nc.declare_dram_parameter("w2", [2 * D, D], BF16, False)
    binT_in = nc.declare_dram_parameter("binT", [128, 2], F32, False)
    b1T_in = nc.declare_dram_parameter("b1T", [128, 4], F32, False)
    b2T_in = nc.declare_dram_parameter("b2T", [128, 2], F32, False)
    cbc_in = nc.declare_dram_parameter("cbc", [128, SEG], F32, False)
    outT_ext = nc.declare_dram_parameter("outT", [D, SEG], F32, True)

    with ExitStack() as es:
        def sem(name):
            return es.enter_context(nc.semaphore(name))

        def sb(name, shape, dt):
            return es.enter_context(nc.sbuf_tensor(name, shape, dt))

        def psum(name, shape, dt):
            return es.enter_context(nc.psum_tensor(name, shape, dt))

        s_cc, s_cf, s_pe, s_fl = sem("cc"), sem("cf"), sem("pe"), sem("fl")
        s_ca, s_cb, s_cd, s_ce = sem("ca"), sem("cb"), sem("cd"), sem("ce")
        s_tr, s_ptc, s_z, s_zc = sem("tr"), sem("ptc"), sem("z"), sem("zc")
        s_h, s_hc, s_o, s_oc = sem("h"), sem("hc"), sem("o"), sem("oc")
        s_do, s_do2 = sem("do"), sem("do2")
        s_cg, s_ch, s_ov = sem("cg"), sem("ch"), sem("ov")
        s_x = [sem(f"x{i}") for i in range(NRING)]

        id2_sb = sb("id2_sb", [128, 2, 128], F8)
        xov_sb = sb("xov_sb", [128, OVT, D], F8)
        sov_sb = sb("sov_sb", [128, OVT, 128], F8)
        idf_sb = sb("idf_sb", [128, 128], F32)
        xbuf = [sb(f"xb{i}", [128, 2 * SLAB_PAIRS, D], F8) for i in range(NRING)]
        winkb = [sb(f"wink{k}", [128, D], BF16) for k in range(2)]
        w1kb = [sb(f"w1k{k}", [128, 2 * D], BF16) for k in range(2)]
        w2kb = [sb(f"w2k{k}", [128, D], BF16) for k in range(4)]
        binT = sb("binT_sb", [128, 2], F32)
        b1T = sb("b1T_sb", [128, 4], F32)
        b2T = sb("b2T_sb", [128, 2], F32)
        cbc = sb("cbc_sb", [128, SEG], F32)
        po = [sb(f"po{w}", [128, D], F32) for w in range(NSLOT)]
        pT = [sb(f"pT{k}", [128, SEG], BF16) for k in range(2)]
        zT = [sb(f"zT{k}", [128, SEG], BF16) for k in range(2)]
        hT = [sb(f"hT{j}", [128, SEG], BF16) for j in range(4)]
        ot = [sb(f"ot{j}", [128, SEG], F32) for j in range(2)]

        # 7 PSUM banks.  A[h]: per chunk parity -- cols 0:128 transposes,
        # 128:256 z, 256:384 o.  hB[h]: h-stage.  pb: stream accumulator
        # ring of 3, so a slot's first matmul never waits on the previous
        # slot's flush.
        pb = [psum(f"pb{i}", [128, D], F32) for i in range(3)]
        A = [psum("A0", [128, 512], F32), psum("A1", [128, 512], F32)]
        hB = [psum("hB0", [128, D], F32), psum("hB1", [128, D], F32)]

        # x slab -> queue: greedy by estimated queue finish time, so slabs
        # arrive roughly in consumption order despite Pool's const preamble.
        # Ring slots are queue-exclusive (SWDGE sems must be Pool-private).
        # SP carries id2+xov up front, Act carries sov, Pool the small consts
        qload = [700.0 + OVT * 256 * 0.386,
                 700.0 + OVT * 128 * 0.386,
                 6800.0]
        qn = [0] * NQ
        queue_of = []
        for (s, t0, np_, first) in slabs:
            qi = min((i for i in range(NQ) if qn[i] < QCAP[i]),
                     key=lambda i: qload[i])
            queue_of.append(qi)
            qn[qi] += 1
            qload[qi] += np_ * 512 * 0.386 + 120
        del qload
        slot_of = [0] * NSLAB
        use_of = [0] * NSLAB      # how many times this slot was used before
        prev_user = [0] * NSLAB   # global index of the slot's previous user
        _count = {}
        _last = {}
        for g, qi in enumerate(queue_of):
            k = _count.get(qi, 0)
            pool_ = QSLOTS[qi]
            slot = pool_[k % len(pool_)]
            slot_of[g] = slot
            use_of[g] = k // len(pool_)
            prev_user[g] = _last.get(slot, -1)
            _last[slot] = g
            _count[qi] = k + 1

        def stream_queue(eng, qi):
            for g in range(NSLAB):
                if queue_of[g] != qi:
                    continue
                s, t0, np_, first = slabs[g]
                if use_of[g]:
                    eng.wait_ge(s_x[slot_of[g]], 16 * use_of[g])
                    eng.wait_ge(s_pe, prev_user[g] + 1)
                eng.dma_start(out=xbuf[slot_of[g]][:, 0:2 * np_, :],
                              in_=xdr_in[:, t0:t0 + 2 * np_, :]
                              ).then_inc(s_x[slot_of[g]], 16)

        with nc.Block(no_gpsimd_drain=True) as block:

            def out_dmas(eng, j, dsem):
                # per-chunk outputs for feature half j, chained on dsem
                for q in range(NCHUNK):
                    w, h = divmod(q, 2)
                    wch = slice(128 * w + 64 * h, 128 * w + 64 * h + 64)
                    eng.wait_ge(s_oc, 2 * q + j + 1)
                    if q:
                        eng.wait_ge(dsem, 16 * q)
                    eng.dma_start(out=outT_ext[j * 128:(j + 1) * 128, wch],
                                  in_=ot[j][:, wch]).then_inc(dsem, 16)
                eng.wait_ge(dsem, 16 * NCHUNK)

            @block.sync
            def _(sp):
                sp.dma_start(out=id2_sb[:, :, :], in_=id2_in[:, :, :]
                             ).then_inc(s_cc, 16)
                sp.dma_start(out=xov_sb[:, :, :], in_=xov_in[:, :, :]
                             ).then_inc(s_cg, 16)
                stream_queue(sp, 0)
                out_dmas(sp, 0, s_do)

            @block.scalar
            def _(a):
                a.dma_start(out=sov_sb[:, :, :], in_=sov_in[:, :, :]
                            ).then_inc(s_ch, 16)
                stream_queue(a, 1)
                out_dmas(a, 1, s_do2)

            @block.gpsimd
            def _(gp):
                # small constants first: independent same-sem chains,
                # interleaved so each link's wait is satisfied on arrival.
                chains = {
                    s_ca: [(winkb[k][:, :], win_in[k * 128:(k + 1) * 128, :])
                           for k in range(2)],
                    s_cb: [(w1kb[k][:, :], w1_in[k * 128:(k + 1) * 128, :])
                           for k in range(2)],
                    s_cd: [(w2kb[k][:, :], w2_in[k * 128:(k + 1) * 128, :])
                           for k in range(4)],
                    s_ce: [(binT[:, :], binT_in[:, :]), (b1T[:, :], b1T_in[:, :]),
                           (b2T[:, :], b2T_in[:, :]), (cbc[:, :], cbc_in[:, :])],
                    s_cf: [(idf_sb[:, :], idf_in[:, :])],
                }
                depth = {}
                for rnd in range(4):
                    for cs, lst in chains.items():
                        if rnd < len(lst):
                            k = depth.get(cs, 0)
                            if k:
                                gp.wait_ge(cs, 16 * k)
                            dst, src = lst[rnd]
                            gp.dma_start(out=dst, in_=src).then_inc(cs, 16)
                            depth[cs] = k + 1
                stream_queue(gp, 2)

            # ---- PE-side MLP stages for chunk (w, h): 64 segment columns.
            # Stages are emitted one slab apart so every wait on a DVE drain
            # is satisfied before PE reaches it (no streaming stalls).
            def _chunk(w, h):
                q = 2 * w + h
                return (q, q % 2,
                        slice(128 * w + 64 * h, 128 * w + 64 * h + 64))

            def mlp_tr(pe, w, h):
                q, cp, wch = _chunk(w, h)
                hsl = slice(64 * h, 64 * h + 64)
                if q == 0:
                    pe.wait_ge(s_cf, 16)   # idf loaded
                if q >= 2:
                    # chunk-parity PSUM reuse: chunk q-2's drains must be done
                    pe.wait_ge(s_ptc, 2 * (q - 1))
                    pe.wait_ge(s_zc, 2 * (q - 1))
                    pe.wait_ge(s_oc, 2 * (q - 1))
                for k in range(2):
                    pe.wait_ge(s_fl, 2 * w + k + 1)
                    pe.transpose(A[cp][:, k * 64:(k + 1) * 64],
                                 po[w][hsl, k * 128:(k + 1) * 128],
                                 idf_sb[hsl, hsl]).then_inc(s_tr, 1)

            def mlp_z(pe, w, h):
                # z = pooled @ W_in  (counts*b_in fused into DVE drain)
                q, cp, wch = _chunk(w, h)
                pe.wait_ge(s_ptc, 2 * q + 2)
                if q == 0:
                    pe.wait_ge(s_ca, 32)
                for j in range(2):
                    jc = slice(j * 128, (j + 1) * 128)
                    dst = A[cp][:, 128 + j * 64:128 + (j + 1) * 64]
                    pe.matmul(dst, winkb[0][:, jc], pT[0][:, wch], start=True, stop=False)
                    pe.matmul(dst, winkb[1][:, jc], pT[1][:, wch],
                              start=False, stop=True).then_inc(s_z, 1)

            def mlp_h(pe, w, h):
                # h = relu(z @ W1 + b1)  (bias+relu fused into DVE drain)
                q, cp, wch = _chunk(w, h)
                pe.wait_ge(s_zc, 2 * q + 2)
                if q == 0:
                    pe.wait_ge(s_cb, 32)
                if q >= 2:
                    pe.wait_ge(s_hc, 4 * (q - 1))
                for j in range(4):
                    jc = slice(j * 128, (j + 1) * 128)
                    dst = hB[cp][:, j * 64:(j + 1) * 64]
                    pe.matmul(dst, w1kb[0][:, jc], zT[0][:, wch], start=True, stop=False)
                    pe.matmul(dst, w1kb[1][:, jc], zT[1][:, wch],
                              start=False, stop=True).then_inc(s_h, 1)

            def mlp_o(pe, w, h):
                # o = h @ W2  (b2 fused into DVE drain)
                q, cp, wch = _chunk(w, h)
                pe.wait_ge(s_hc, 4 * q + 4)
                if q == 0:
                    pe.wait_ge(s_cd, 64)
                for j in range(2):
                    jc = slice(j * 128, (j + 1) * 128)
                    dst = A[cp][:, 256 + j * 64:256 + (j + 1) * 64]
                    for i in range(4):
                        mm = pe.matmul(dst, w2kb[i][:, jc], hT[i][:, wch],
                                       start=(i == 0), stop=(i == 3))
                    mm.then_inc(s_o, 1)

            @block.tensor
            def _(pe):
                pe.wait_ge(s_cc, 16)   # id2 loaded
                # clock warm-up: the PE p-state ramps to full speed only after
                # 3us of continuous execution; burn idle pre-stream time on
                # dummy matmuls so slab 0 is processed at full rate (and PE
                # carries a small lag buffer so arrivals stay ahead of it).
                for _ in range(WARMUP):
                    pe.matmul(hB[1][:, 0:128], id2_sb[:, :, :], id2_sb[:, :, :],
                              start=True, stop=True, perf_mode=DR)
                # stage schedule: window s-1's MLP stages spread over the
                # first 4 slabs of slot s; o(s-1, 1) lands after the next
                # slot's first slab so its s_hc wait is pre-satisfied.
                stage_after = {}
                for g, (s, t0, np_, first) in enumerate(slabs):
                    if first and s >= 1:
                        last = cumslabs[s] - 1
                        p = s - 1
                        stage_after.setdefault(g, []).append((mlp_tr, p, 0))
                        stage_after.setdefault(min(g + 1, last), []).extend(
                            [(mlp_z, p, 0), (mlp_tr, p, 1)])
                        stage_after.setdefault(min(g + 2, last), []).extend(
                            [(mlp_h, p, 0), (mlp_z, p, 1)])
                        stage_after.setdefault(min(g + 3, last), []).extend(
                            [(mlp_o, p, 0), (mlp_h, p, 1), (mlp_o, p, 1)])
                for g, (s, t0, np_, first) in enumerate(slabs):
                    pe.wait_ge(s_x[slot_of[g]], 16 * (use_of[g] + 1))
                    if first and s >= 3:
                        pe.wait_ge(s_fl, 2 * (s - 2))
                    k0 = (t0 - sum(TP[:s])) // 2
                    for i in range(np_):
                        kk = k0 + i
                        mm = pe.matmul(pb[s % 3][:, 0:D], id2_sb[:, :, :],
                                       xbuf[slot_of[g]][:, 2 * i:2 * i + 2, :],
                                       start=(kk == 0), stop=False,
                                       perf_mode=DR)
                        if i == np_ - 1:
                            mm.then_inc(s_pe, 1)
                    if g == cumslabs[s] - 1:
                        # overflow one-hot pairs close the accumulation group
                        if s == 0:
                            pe.wait_ge(s_cg, 16)
                            pe.wait_ge(s_ch, 16)
                        ob = 2 * sum(NOV[:s])
                        for k in range(NOV[s]):
                            mm = pe.matmul(
                                pb[s % 3][:, 0:D],
                                sov_sb[:, ob + 2 * k:ob + 2 * k + 2, :],
                                xov_sb[:, ob + 2 * k:ob + 2 * k + 2, :],
                                start=False, stop=(k == NOV[s] - 1),
                                perf_mode=DR)
                        mm.then_inc(s_ov, 1)
                    for (fn, w, h) in stage_after.get(g, []):
                        fn(pe, w, h)
                # tail: window 3 zippered
                p = NSLOT - 1
                for fn in (mlp_tr, mlp_z, mlp_h, mlp_o):
                    fn(pe, p, 0)
                    fn(pe, p, 1)

            # ---- DVE-side drains, stage granular
            def dve_ptc(v, w, h):
                q, cp, wch = _chunk(w, h)
                v.wait_ge(s_tr, 2 * q + 2)
                for k in range(2):
                    v.tensor_copy(pT[k][:, wch],
                                  A[cp][:, k * 64:(k + 1) * 64]).then_inc(s_ptc, 1)

            def dve_zc(v, w, h):
                q, cp, wch = _chunk(w, h)
                if q == 0:
                    v.wait_ge(s_ce, 64)   # binT/b1T/b2T/cbc loaded
                v.wait_ge(s_z, 2 * q + 2)
                for j in range(2):
                    # zT = zP + b_in[j-block] (x) counts
                    v.scalar_tensor_tensor(
                        zT[j][:, wch], cbc[:, wch], binT[:, j:j + 1],
                        A[cp][:, 128 + j * 64:128 + (j + 1) * 64],
                        MULT, ADD).then_inc(s_zc, 1)

            def dve_hc(v, w, h):
                q, cp, wch = _chunk(w, h)
                v.wait_ge(s_h, 4 * q + 4)
                for j in range(4):
                    # hT = relu(hP + b1[j-block])
                    v.tensor_scalar(hT[j][:, wch],
                                    hB[cp][:, j * 64:(j + 1) * 64],
                                    b1T[:, j:j + 1], 0.0, ADD, MAX).then_inc(s_hc, 1)

            def dve_oc(v, w, h):
                q, cp, wch = _chunk(w, h)
                v.wait_ge(s_o, 2 * q + 2)
                for j in range(2):
                    # ot = oP + b2[j-block]
                    v.tensor_scalar(ot[j][:, wch],
                                    A[cp][:, 256 + j * 64:256 + (j + 1) * 64],
                                    b2T[:, j:j + 1], None, ADD).then_inc(s_oc, 1)

            @block.vector
            def _(v):
                # drains for window w-1 that gate PE's pre-flush stages MUST
                # precede flush(w); hc(w-1,1)/oc(w-1,*) only gate stages PE
                # reaches after flush(w), so flush slots in between (it waits
                # only on s_pe, which PE raises before those stages).
                def flush(w):
                    v.wait_ge(s_pe, cumslabs[w])
                    v.wait_ge(s_ov, w + 1)
                    for k in range(2):
                        v.tensor_copy(po[w][:, k * 128:(k + 1) * 128],
                                      pb[w % 3][:, k * 128:(k + 1) * 128]
                                      ).then_inc(s_fl, 1)
                flush(0)
                for w in range(1, NSLOT + 1):
                    p = w - 1
                    dve_ptc(v, p, 0)
                    dve_ptc(v, p, 1)
                    dve_zc(v, p, 0)
                    dve_zc(v, p, 1)
                    dve_hc(v, p, 0)
                    if w < NSLOT:
                        flush(w)
                    dve_hc(v, p, 1)
                    dve_oc(v, p, 0)
                    dve_oc(v, p, 1)

    return nc


def _quantize_feedback(x, sizes, starts, order):
    """fp8 e4m3 with per-(segment, column) sigma-delta error feedback."""
    xq = np.empty(x.shape, dtype=NPF8)
    # process segments in descending-size order so live set is a prefix
    sz_d = sizes[order]                       # descending
    st_d = starts[order]
    carry = np.zeros((NSEG, D), np.float32)
    maxlen = int(sz_d[0])
    for r in range(maxlen):
        m = int(np.searchsorted(-sz_d, -(r + 1), side="right"))
        rows = st_d[:m] + r
        acc = x[rows] + carry[:m]
        q = acc.astype(NPF8)
        xq[rows] = q
        carry[:m] = acc - q.astype(np.float32)
    return xq


def _plan(batch):
    sizes = np.bincount(batch, minlength=NSEG).astype(np.int64)
    starts = np.concatenate([[0], np.cumsum(sizes)])[:-1]
    order = np.argsort(-sizes, kind="stable")
    # per slot-group: cap the stream at t_s rows/segment and push the excess
    # of oversized segments into one-hot overflow pairs; choose t_s to
    # minimize per-partition stream bytes (256/row regular, 768/ov-pair-row
    # since overflow also carries its one-hot lhsT)
    TP, NOV = [], []
    for s in range(NSLOT):
        grp = sizes[order[1024 * s:1024 * (s + 1)]]            # descending
        tmax = int(grp[0] + 1) // 2 * 2
        best = (None, None)
        for t in range(int(grp.min()) // 2 * 2, tmax + 2, 2):
            ov_core = [
                int(np.maximum(grp[c::8] - t, 0).sum()) for c in range(N_CORES)
            ]
            nov = max(1, -(-max(ov_core) // 256))
            cost = t * 256 + nov * 2 * (256 + 128)
            if best[0] is None or cost < best[0]:
                best = (cost, (t, nov))
        TP.append(best[1][0])
        NOV.append(best[1][1])
    return sizes, starts, order, TP, NOV


def prepare_inputs(inputs):
    """Host-side shard plan: returns (TP, per_core input maps, core_segs)."""
    x = np.ascontiguousarray(np.asarray(inputs["x"], np.float32))
    batch = np.asarray(inputs["batch"]).astype(np.int64)
    W_in = np.asarray(inputs["W_in"], np.float32)
    b_in = np.asarray(inputs["b_in"], np.float32).reshape(1, D)
    W1 = np.asarray(inputs["W1"], np.float32)
    b1 = np.asarray(inputs["b1"], np.float32).reshape(1, 2 * D)
    W2 = np.asarray(inputs["W2"], np.float32)
    b2 = np.asarray(inputs["b2"], np.float32).reshape(1, D)

    sizes, starts, order, TP, NOV = _plan(batch)
    TOT = sum(TP)
    OVT = 2 * sum(NOV)
    xq = _quantize_feedback(x, sizes, starts, order)
    xq_pad = np.concatenate([xq, np.zeros((1, D), NPF8)])

    id2 = np.stack([np.eye(128, dtype=np.float32)] * 2, axis=1).astype(NPF8)
    idf = np.eye(128, dtype=np.float32)
    shared = dict(
        id2=id2, idf=idf,
        win=W_in.astype(NPBF), w1=W1.astype(NPBF), w2=W2.astype(NPBF),
        binT=np.ascontiguousarray(b_in.reshape(2, 128).T),
        b1T=np.ascontiguousarray(b1.reshape(4, 128).T),
        b2T=np.ascontiguousarray(b2.reshape(2, 128).T),
    )

    per_core = []
    core_segs = []
    for c in range(N_CORES):
        idx = np.full((128, TOT), N, np.int64)
        ovx = np.full((128, OVT), N, np.int64)    # overflow row gather plan
        sov = np.zeros((128, OVT, 128), np.float32)
        segs_c = np.empty(SEG, np.int64)
        off = 0
        ovoff = 0
        for s in range(NSLOT):
            segs = order[1024 * s + c:1024 * (s + 1):8]   # strided: balanced
            segs_c[128 * s:128 * s + 128] = segs
            t = TP[s]
            l = 0                                  # overflow linear cursor
            for p in range(128):
                n = int(sizes[segs[p]])
                keep = min(n, t)
                idx[p, off:off + keep] = starts[segs[p]] + np.arange(keep)
                if n > t:
                    r = starts[segs[p]] + np.arange(t, n)
                    li = l + np.arange(n - t)
                    ovx[li % 128, ovoff + li // 128] = r
                    sov[li % 128, ovoff + li // 128, p] = 1.0
                    l += n - t
            assert l <= NOV[s] * 256
            off += t
            ovoff += 2 * NOV[s]
        xdr = xq_pad[idx.reshape(-1)].reshape(128, TOT, D)
        xov = xq_pad[ovx.reshape(-1)].reshape(128, OVT, D)
        crow = sizes[segs_c].astype(np.float32).reshape(1, SEG)
        m = dict(shared)
        m.update(xdr=xdr, xov=xov, sov=sov.astype(NPF8),
                 cbc=np.repeat(crow, 128, axis=0))
        per_core.append(m)
        core_segs.append(segs_c)
    return TP, NOV, per_core, core_segs


def kernel(**inputs):
    TP, NOV, per_core, core_segs = prepare_inputs(inputs)
    nc = build_program(TP, NOV)
    res = run_bass_kernel_spmd(nc, per_core, list(range(N_CORES)))

    out = np.empty((NSEG, D), np.float32)
    for c in range(N_CORES):
        out[core_segs[c]] = res.results[c]["outT"].T
    return out


# revision 66
# speedup vs baseline: 1.0301x; 1.0301x over previous
"""Trainium2 Bass kernel for segment_reduce MLP (nn_HeadSemantic_35983236006251).

Math shortcut: Linear commutes with segment_sum, so
    pooled = segment_sum(x @ W_in + b_in) = segment_sum(x) @ W_in + counts * b_in
and the kernel reduces to memory-bound streaming of x into per-segment sums,
followed by a tiny MLP on [4096, 256].

Design (vs. the one-hot baseline):
  * x is streamed in fp8 (e4m3) instead of fp32 -- 4x less HBM traffic.
    Host-side quantization uses error feedback (sigma-delta) along each
    (segment, column) chain, so the device's exact-fp32 PSUM accumulation sees
    a segment-sum error of ~1 quantization step instead of ~sqrt(n) steps.
    Measured end-to-end rel err ~4e-3 (gate is 2e-2).
  * No per-tile one-hot build at all: segments are sorted by size on the host
    and assigned one-per-partition; x is re-laid-out in DRAM as per-partition
    row streams.  The segment-sum is then a PSUM accumulation with a CONSTANT
    doubled-identity lhsT in fp8 DoubleRow mode (2 tiles of 128 rows per
    matmul).
  * x is DMAed in big per-partition-contiguous slabs (16 KB/partition),
    spread over THREE DGE queues (SP / Activation / Pool, greedy-balanced)
    so the descriptor rings never throttle the HBM stream; Pool's queue
    front carries the small constants, SP/Act the overflow tensors.
  * Oversized segments are capped at a per-slot threshold; their excess rows
    go through a few host-built one-hot DoubleRow matmuls appended to each
    window's accumulation group (padding overhead ~6% -> ~1%).
  * The MLP runs per 64-segment chunk as soon as its window's pooled sums
    flush, overlapped with streaming of later windows; weights/activations in
    bf16 (PSUM accumulation fp32); biases fused into the DVE PSUM->SBUF
    copies (per-partition tensor_scalar add / add+relu), the counts*b_in term
    via scalar_tensor_tensor against a host-broadcast counts plane.  PSUM
    scratch alternates by chunk parity so chunk q+1's matmuls overlap chunk
    q's drain copies; only the final chunk's drain is tail latency.

Sharding: segments sorted by size desc; window w (of 32) = segments
[128w, 128w+128); slot-group s = windows [8s, 8s+8); core c takes window
8s + c of each group.  All cores share one SPMD program whose per-slot tile
counts are the group maxima.
"""

import sys
import numpy as np
import ml_dtypes
from contextlib import ExitStack

sys.path.insert(0, "/opt/trn_rl_repo")

import concourse.bass as bass
from concourse import mybir
from concourse.bass_utils import run_bass_kernel_spmd

N = 1_000_000
D = 256
NSEG = 4096
N_CORES = 8
NSLOT = 4                  # windows per core
SEG = NSLOT * 128          # segments per core
NCHUNK = 2 * NSLOT         # 64-segment MLP chunks
F32 = mybir.dt.float32
BF16 = mybir.dt.bfloat16
F8 = mybir.dt.float8e4
NPF8 = ml_dtypes.float8_e4m3
NPBF = ml_dtypes.bfloat16
SLAB_PAIRS = 32            # row-tile pairs per DMA slab (64 tiles, 16KB/part)
NQ = 3                     # x DMA queues (SP, Act, Pool)
QSLOTS = [[0, 1, 2], [3, 4, 5], [6, 7, 8]]        # queue-exclusive xbuf slots
QCAP = [99, 99, 99]
NRING = 9
DR = mybir.MatmulPerfMode.DoubleRow
ADD = mybir.AluOpType.add
MAX = mybir.AluOpType.max
MULT = mybir.AluOpType.mult
NCONST = 12                # MLP const DMAs on s_c


def _slab_plan(TP):
    """TP = tiles per slot (even).  Each slab is
    (slot, dram_tile0, npairs, first_of_slot)."""
    slabs = []
    cumslabs = []
    base = 0
    for s, tp in enumerate(TP):
        pairs = tp // 2
        k = 0
        while k < pairs:
            np_ = min(SLAB_PAIRS, pairs - k)
            slabs.append((s, base + 2 * k, np_, k == 0))
            k += np_
        cumslabs.append(len(slabs))
        base += tp
    return slabs, cumslabs


def build_program(TP, NOV):
    nc = bass.Bass()
    TOT = sum(TP)
    OVT = 2 * sum(NOV)
    PAIRS = [tp // 2 for tp in TP]
    slabs, cumslabs = _slab_plan(TP)
    NSLAB = len(slabs)

    xdr_in = nc.declare_dram_parameter("xdr", [128, TOT, D], F8, False)
    xov_in = nc.declare_dram_parameter("xov", [128, OVT, D], F8, False)
    sov_in = nc.declare_dram_parameter("sov", [128, OVT, 128], F8, False)
    id2_in = nc.declare_dram_parameter("id2", [128, 2, 128], F8, False)
    idf_in = nc.declare_dram_parameter("idf", [128, 128], F32, False)
    win_in = nc.declare_dram_parameter("win", [D, D], BF16, False)
    w1_in = nc.declare_dram_parameter("w1", [D, 2 * D], BF16, False)
    w2_in = nc.declare_dram_parameter("w2", [2 * D, D], BF16, False)
    binT_in = nc.declare_dram_parameter("binT", [128, 2], F32, False)
    b1T_in = nc.declare_dram_parameter("b1T", [128, 4], F32, False)
    b2T_in = nc.declare_dram_parameter("b2T", [128, 2], F32, False)
    cbc_in = nc.declare_dram_parameter("cbc", [128, SEG], F32, False)
    outT_ext = nc.declare_dram_parameter("outT", [D, SEG], F32, True)

    with ExitStack() as es:
        def sem(name):
            return es.enter_context(nc.semaphore(name))

        def sb(name, shape, dt):
            return es.enter_context(nc.sbuf_tensor(name, shape, dt))

        def psum(name, shape, dt):
            return es.enter_context(nc.psum_tensor(name, shape, dt))

        s_cc, s_cf, s_pe, s_fl = sem("cc"), sem("cf"), sem("pe"), sem("fl")
        s_ca, s_cb, s_cd, s_ce = sem("ca"), sem("cb"), sem("cd"), sem("ce")
        s_tr, s_ptc, s_z, s_zc = sem("tr"), sem("ptc"), sem("z"), sem("zc")
        s_h, s_hc, s_o, s_oc = sem("h"), sem("hc"), sem("o"), sem("oc")
        s_do, s_do2 = sem("do"), sem("do2")
        s_cg, s_ch, s_ov = sem("cg"), sem("ch"), sem("ov")
        s_x = [sem(f"x{i}") for i in range(NRING)]

        id2_sb = sb("id2_sb", [128, 2, 128], F8)
        xov_sb = sb("xov_sb", [128, OVT, D], F8)
        sov_sb = sb("sov_sb", [128, OVT, 128], F8)
        idf_sb = sb("idf_sb", [128, 128], F32)
        xbuf = [sb(f"xb{i}", [128, 2 * SLAB_PAIRS, D], F8) for i in range(NRING)]
        winkb = [sb(f"wink{k}", [128, D], BF16) for k in range(2)]
        w1kb = [sb(f"w1k{k}", [128, 2 * D], BF16) for k in range(2)]
        w2kb = [sb(f"w2k{k}", [128, D], BF16) for k in range(4)]
        binT = sb("binT_sb", [128, 2], F32)
        b1T = sb("b1T_sb", [128, 4], F32)
        b2T = sb("b2T_sb", [128, 2], F32)
        cbc = sb("cbc_sb", [128, SEG], F32)
        po = [sb(f"po{w}", [128, D], F32) for w in range(NSLOT)]
        pT = [sb(f"pT{k}", [128, SEG], BF16) for k in range(2)]
        zT = [sb(f"zT{k}", [128, SEG], BF16) for k in range(2)]
        hT = [sb(f"hT{j}", [128, SEG], BF16) for j in range(4)]
        ot = [sb(f"ot{j}", [128, SEG], F32) for j in range(2)]

        # 7 PSUM banks.  A[h]: per chunk parity -- cols 0:128 transposes,
        # 128:256 z, 256:384 o.  hB[h]: h-stage.  pb: stream accumulator
        # ring of 3, so a slot's first matmul never waits on the previous
        # slot's flush.
        pb = [psum(f"pb{i}", [128, D], F32) for i in range(3)]
        A = [psum("A0", [128, 512], F32), psum("A1", [128, 512], F32)]
        A2 = psum("A2", [128, 512], F32)   # fresh scratch for chunk 6 (tail)
        hB = [psum("hB0", [128, D], F32), psum("hB1", [128, D], F32)]

        def A_of(q):
            return A2 if q == 6 else A[q % 2]

        # x slab -> queue: greedy by estimated queue finish time, so slabs
        # arrive roughly in consumption order despite Pool's const preamble.
        # Ring slots are queue-exclusive (SWDGE sems must be Pool-private).
        # SP carries id2+xov up front, Act carries sov, Pool the small consts
        qload = [700.0 + OVT * 256 * 0.386,
                 700.0 + OVT * 128 * 0.386,
                 6800.0]
        qn = [0] * NQ
        queue_of = []
        for (s, t0, np_, first) in slabs:
            qi = min((i for i in range(NQ) if qn[i] < QCAP[i]),
                     key=lambda i: qload[i])
            queue_of.append(qi)
            qn[qi] += 1
            qload[qi] += np_ * 512 * 0.386 + 120
        del qload
        slot_of = [0] * NSLAB
        use_of = [0] * NSLAB      # how many times this slot was used before
        prev_user = [0] * NSLAB   # global index of the slot's previous user
        _count = {}
        _last = {}
        for g, qi in enumerate(queue_of):
            k = _count.get(qi, 0)
            pool_ = QSLOTS[qi]
            slot = pool_[k % len(pool_)]
            slot_of[g] = slot
            use_of[g] = k // len(pool_)
            prev_user[g] = _last.get(slot, -1)
            _last[slot] = g
            _count[qi] = k + 1

        def stream_queue(eng, qi):
            for g in range(NSLAB):
                if queue_of[g] != qi:
                    continue
                s, t0, np_, first = slabs[g]
                if use_of[g]:
                    eng.wait_ge(s_x[slot_of[g]], 16 * use_of[g])
                    eng.wait_ge(s_pe, prev_user[g] + 1)
                eng.dma_start(out=xbuf[slot_of[g]][:, 0:2 * np_, :],
                              in_=xdr_in[:, t0:t0 + 2 * np_, :]
                              ).then_inc(s_x[slot_of[g]], 16)

        with nc.Block(no_gpsimd_drain=True) as block:

            def out_dmas(eng, j, dsem):
                # per-chunk outputs for feature half j, chained on dsem
                for q in range(NCHUNK):
                    w, h = divmod(q, 2)
                    wch = slice(128 * w + 64 * h, 128 * w + 64 * h + 64)
                    eng.wait_ge(s_oc, 2 * q + j + 1)
                    if q:
                        eng.wait_ge(dsem, 16 * q)
                    eng.dma_start(out=outT_ext[j * 128:(j + 1) * 128, wch],
                                  in_=ot[j][:, wch]).then_inc(dsem, 16)
                eng.wait_ge(dsem, 16 * NCHUNK)

            @block.sync
            def _(sp):
                sp.dma_start(out=id2_sb[:, :, :], in_=id2_in[:, :, :]
                             ).then_inc(s_cc, 16)
                sp.dma_start(out=xov_sb[:, :, :], in_=xov_in[:, :, :]
                             ).then_inc(s_cg, 16)
                stream_queue(sp, 0)
                out_dmas(sp, 0, s_do)

            @block.scalar
            def _(a):
                a.dma_start(out=sov_sb[:, :, :], in_=sov_in[:, :, :]
                            ).then_inc(s_ch, 16)
                stream_queue(a, 1)
                out_dmas(a, 1, s_do2)

            @block.gpsimd
            def _(gp):
                # small constants first: independent same-sem chains,
                # interleaved so each link's wait is satisfied on arrival.
                chains = {
                    s_ca: [(winkb[k][:, :], win_in[k * 128:(k + 1) * 128, :])
                           for k in range(2)],
                    s_cb: [(w1kb[k][:, :], w1_in[k * 128:(k + 1) * 128, :])
                           for k in range(2)],
                    s_cd: [(w2kb[k][:, :], w2_in[k * 128:(k + 1) * 128, :])
                           for k in range(4)],
                    s_ce: [(binT[:, :], binT_in[:, :]), (b1T[:, :], b1T_in[:, :]),
                           (b2T[:, :], b2T_in[:, :]), (cbc[:, :], cbc_in[:, :])],
                    s_cf: [(idf_sb[:, :], idf_in[:, :])],
                }
                depth = {}
                for rnd in range(4):
                    for cs, lst in chains.items():
                        if rnd < len(lst):
                            k = depth.get(cs, 0)
                            if k:
                                gp.wait_ge(cs, 16 * k)
                            dst, src = lst[rnd]
                            gp.dma_start(out=dst, in_=src).then_inc(cs, 16)
                            depth[cs] = k + 1
                stream_queue(gp, 2)

            # ---- PE-side MLP stages for chunk (w, h): 64 segment columns.
            # Stages are emitted one slab apart so every wait on a DVE drain
            # is satisfied before PE reaches it (no streaming stalls).
            def _chunk(w, h):
                q = 2 * w + h
                return (q, q % 2,
                        slice(128 * w + 64 * h, 128 * w + 64 * h + 64))

            def mlp_tr(pe, w, h):
                q, cp, wch = _chunk(w, h)
                hsl = slice(64 * h, 64 * h + 64)
                if q == 0:
                    pe.wait_ge(s_cf, 16)   # idf loaded
                if q >= 2 and q != 6:
                    # chunk-parity PSUM reuse: chunk q-2's drains must be done
                    pe.wait_ge(s_ptc, 2 * (q - 1))
                    pe.wait_ge(s_zc, 2 * (q - 1))
                    pe.wait_ge(s_oc, 2 * (q - 1))
                for k in range(2):
                    pe.wait_ge(s_fl, 2 * w + k + 1)
                    pe.transpose(A_of(q)[:, k * 64:(k + 1) * 64],
                                 po[w][hsl, k * 128:(k + 1) * 128],
                                 idf_sb[hsl, hsl]).then_inc(s_tr, 1)

            def mlp_z(pe, w, h):
                # z = pooled @ W_in  (counts*b_in fused into DVE drain)
                q, cp, wch = _chunk(w, h)
                pe.wait_ge(s_ptc, 2 * q + 2)
                if q == 0:
                    pe.wait_ge(s_ca, 32)
                for j in range(2):
                    jc = slice(j * 128, (j + 1) * 128)
                    dst = A_of(q)[:, 128 + j * 64:128 + (j + 1) * 64]
                    pe.matmul(dst, winkb[0][:, jc], pT[0][:, wch], start=True, stop=False)
                    pe.matmul(dst, winkb[1][:, jc], pT[1][:, wch],
                              start=False, stop=True).then_inc(s_z, 1)

            def mlp_h(pe, w, h):
                # h = relu(z @ W1 + b1)  (bias+relu fused into DVE drain)
                q, cp, wch = _chunk(w, h)
                pe.wait_ge(s_zc, 2 * q + 2)
                if q == 0:
                    pe.wait_ge(s_cb, 32)
                if q >= 2:
                    pe.wait_ge(s_hc, 4 * (q - 1))
                for j in range(4):
                    jc = slice(j * 128, (j + 1) * 128)
                    dst = hB[cp][:, j * 64:(j + 1) * 64]
                    pe.matmul(dst, w1kb[0][:, jc], zT[0][:, wch], start=True, stop=False)
                    pe.matmul(dst, w1kb[1][:, jc], zT[1][:, wch],
                              start=False, stop=True).then_inc(s_h, 1)

            def mlp_o(pe, w, h):
                # o = h @ W2  (b2 fused into DVE drain)
                q, cp, wch = _chunk(w, h)
                pe.wait_ge(s_hc, 4 * q + 4)
                if q == 0:
                    pe.wait_ge(s_cd, 64)
                for j in range(2):
                    jc = slice(j * 128, (j + 1) * 128)
                    dst = A_of(q)[:, 256 + j * 64:256 + (j + 1) * 64]
                    for i in range(4):
                        mm = pe.matmul(dst, w2kb[i][:, jc], hT[i][:, wch],
                                       start=(i == 0), stop=(i == 3))
                    mm.then_inc(s_o, 1)

            @block.tensor
            def _(pe):
                pe.wait_ge(s_cc, 16)   # id2 loaded
                # clock warm-up: the PE p-state ramps to full speed only after
                # 3us of continuous execution; burn idle pre-stream time on
                # dummy matmuls so slab 0 is processed at full rate (and PE
                # carries a small lag buffer so arrivals stay ahead of it).
                for _ in range(WARMUP):
                    pe.matmul(hB[1][:, 0:128], id2_sb[:, :, :], id2_sb[:, :, :],
                              start=True, stop=True, perf_mode=DR)
                # stage schedule: window s-1's MLP stages spread over the
                # first 4 slabs of slot s, one stage (both chunks) per slab
                stage_after = {}
                for g, (s, t0, np_, first) in enumerate(slabs):
                    if first and s >= 1:
                        last = cumslabs[s] - 1
                        p = s - 1
                        for d, fn in enumerate((mlp_tr, mlp_z, mlp_h, mlp_o)):
                            stage_after.setdefault(min(g + d, last), []).extend(
                                [(fn, p, 0), (fn, p, 1)])
                for g, (s, t0, np_, first) in enumerate(slabs):
                    pe.wait_ge(s_x[slot_of[g]], 16 * (use_of[g] + 1))
                    if first and s >= 3:
                        pe.wait_ge(s_fl, 2 * (s - 2))
                    k0 = (t0 - sum(TP[:s])) // 2
                    for i in range(np_):
                        kk = k0 + i
                        mm = pe.matmul(pb[s % 3][:, 0:D], id2_sb[:, :, :],
                                       xbuf[slot_of[g]][:, 2 * i:2 * i + 2, :],
                                       start=(kk == 0), stop=False,
                                       perf_mode=DR)
                        if i == np_ - 1:
                            mm.then_inc(s_pe, 1)
                    if g == cumslabs[s] - 1:
                        # overflow one-hot pairs close the accumulation group
                        if s == 0:
                            pe.wait_ge(s_cg, 16)
                            pe.wait_ge(s_ch, 16)
                        ob = 2 * sum(NOV[:s])
                        for k in range(NOV[s]):
                            mm = pe.matmul(
                                pb[s % 3][:, 0:D],
                                sov_sb[:, ob + 2 * k:ob + 2 * k + 2, :],
                                xov_sb[:, ob + 2 * k:ob + 2 * k + 2, :],
                                start=False, stop=(k == NOV[s] - 1),
                                perf_mode=DR)
                        mm.then_inc(s_ov, 1)
                    for (fn, w, h) in stage_after.get(g, []):
                        fn(pe, w, h)
                # tail: window 3 zippered
                p = NSLOT - 1
                for fn in (mlp_tr, mlp_z, mlp_h, mlp_o):
                    fn(pe, p, 0)
                    fn(pe, p, 1)

            # ---- DVE-side drains, stage granular
            def dve_ptc(v, w, h):
                q, cp, wch = _chunk(w, h)
                v.wait_ge(s_tr, 2 * q + 2)
                for k in range(2):
                    v.tensor_copy(pT[k][:, wch],
                                  A_of(q)[:, k * 64:(k + 1) * 64]).then_inc(s_ptc, 1)

            def dve_zc(v, w, h):
                q, cp, wch = _chunk(w, h)
                if q == 0:
                    v.wait_ge(s_ce, 64)   # binT/b1T/b2T/cbc loaded
                v.wait_ge(s_z, 2 * q + 2)
                for j in range(2):
                    # zT = zP + b_in[j-block] (x) counts
                    v.scalar_tensor_tensor(
                        zT[j][:, wch], cbc[:, wch], binT[:, j:j + 1],
                        A_of(q)[:, 128 + j * 64:128 + (j + 1) * 64],
                        MULT, ADD).then_inc(s_zc, 1)

            def dve_hc(v, w, h):
                q, cp, wch = _chunk(w, h)
                v.wait_ge(s_h, 4 * q + 4)
                for j in range(4):
                    # hT = relu(hP + b1[j-block])
                    v.tensor_scalar(hT[j][:, wch],
                                    hB[cp][:, j * 64:(j + 1) * 64],
                                    b1T[:, j:j + 1], 0.0, ADD, MAX).then_inc(s_hc, 1)

            def dve_oc(v, w, h):
                q, cp, wch = _chunk(w, h)
                v.wait_ge(s_o, 2 * q + 2)
                for j in range(2):
                    # ot = oP + b2[j-block]
                    v.tensor_scalar(ot[j][:, wch],
                                    A_of(q)[:, 256 + j * 64:256 + (j + 1) * 64],
                                    b2T[:, j:j + 1], None, ADD).then_inc(s_oc, 1)

            @block.vector
            def _(v):
                # drains for window w-1 that gate PE's pre-flush stages MUST
                # precede flush(w); hc(w-1,1)/oc(w-1,*) only gate stages PE
                # reaches after flush(w), so flush slots in between (it waits
                # only on s_pe, which PE raises before those stages).
                def flush(w):
                    v.wait_ge(s_pe, cumslabs[w])
                    v.wait_ge(s_ov, w + 1)
                    for k in range(2):
                        v.tensor_copy(po[w][:, k * 128:(k + 1) * 128],
                                      pb[w % 3][:, k * 128:(k + 1) * 128]
                                      ).then_inc(s_fl, 1)
                flush(0)
                for w in range(1, NSLOT + 1):
                    p = w - 1
                    dve_ptc(v, p, 0)
                    dve_ptc(v, p, 1)
                    dve_zc(v, p, 0)
                    dve_zc(v, p, 1)
                    dve_hc(v, p, 0)
                    if w < NSLOT:
                        flush(w)
                    dve_hc(v, p, 1)
                    dve_oc(v, p, 0)
                    dve_oc(v, p, 1)

    return nc


def _quantize_feedback(x, sizes, starts, order):
    """fp8 e4m3 with per-(segment, column) sigma-delta error feedback."""
    xq = np.empty(x.shape, dtype=NPF8)
    # process segments in descending-size order so live set is a prefix
    sz_d = sizes[order]                       # descending
    st_d = starts[order]
    carry = np.zeros((NSEG, D), np.float32)
    maxlen = int(sz_d[0])
    for r in range(maxlen):
        m = int(np.searchsorted(-sz_d, -(r + 1), side="right"))
        rows = st_d[:m] + r
        acc = x[rows] + carry[:m]
        q = acc.astype(NPF8)
        xq[rows] = q
        carry[:m] = acc - q.astype(np.float32)
    return xq


def _plan(batch):
    sizes = np.bincount(batch, minlength=NSEG).astype(np.int64)
    starts = np.concatenate([[0], np.cumsum(sizes)])[:-1]
    order = np.argsort(-sizes, kind="stable")
    # per slot-group: cap the stream at t_s rows/segment and push the excess
    # of oversized segments into one-hot overflow pairs; choose t_s to
    # minimize per-partition stream bytes (256/row regular, 768/ov-pair-row
    # since overflow also carries its one-hot lhsT)
    TP, NOV = [], []
    for s in range(NSLOT):
        grp = sizes[order[1024 * s:1024 * (s + 1)]]            # descending
        tmax = int(grp[0] + 1) // 2 * 2
        best = (None, None)
        for t in range(int(grp.min()) // 2 * 2, tmax + 2, 2):
            ov_core = [
                int(np.maximum(grp[c::8] - t, 0).sum()) for c in range(N_CORES)
            ]
            nov = max(1, -(-max(ov_core) // 256))
            cost = t * 256 + nov * 2 * (256 + 128)
            if best[0] is None or cost < best[0]:
                best = (cost, (t, nov))
        TP.append(best[1][0])
        NOV.append(best[1][1])
    return sizes, starts, order, TP, NOV


def prepare_inputs(inputs):
    """Host-side shard plan: returns (TP, per_core input maps, core_segs)."""
    x = np.ascontiguousarray(np.asarray(inputs["x"], np.float32))
    batch = np.asarray(inputs["batch"]).astype(np.int64)
    W_in = np.asarray(inputs["W_in"], np.float32)
    b_in = np.asarray(inputs["b_in"], np.float32).reshape(1, D)
    W1 = np.asarray(inputs["W1"], np.float32)
    b1 = np.asarray(inputs["b1"], np.float32).reshape(1, 2 * D)
    W2 = np.asarray(inputs["W2"], np.float32)
    b2 = np.asarray(inputs["b2"], np.float32).reshape(1, D)

    sizes, starts, order, TP, NOV = _plan(batch)
    TOT = sum(TP)
    OVT = 2 * sum(NOV)
    xq = _quantize_feedback(x, sizes, starts, order)
    xq_pad = np.concatenate([xq, np.zeros((1, D), NPF8)])

    id2 = np.stack([np.eye(128, dtype=np.float32)] * 2, axis=1).astype(NPF8)
    idf = np.eye(128, dtype=np.float32)
    shared = dict(
        id2=id2, idf=idf,
        win=W_in.astype(NPBF), w1=W1.astype(NPBF), w2=W2.astype(NPBF),
        binT=np.ascontiguousarray(b_in.reshape(2, 128).T),
        b1T=np.ascontiguousarray(b1.reshape(4, 128).T),
        b2T=np.ascontiguousarray(b2.reshape(2, 128).T),
    )

    per_core = []
    core_segs = []
    for c in range(N_CORES):
        idx = np.full((128, TOT), N, np.int64)
        ovx = np.full((128, OVT), N, np.int64)    # overflow row gather plan
        sov = np.zeros((128, OVT, 128), np.float32)
        segs_c = np.empty(SEG, np.int64)
        off = 0
        ovoff = 0
        for s in range(NSLOT):
            segs = order[1024 * s + c:1024 * (s + 1):8]   # strided: balanced
            segs_c[128 * s:128 * s + 128] = segs
            t = TP[s]
            l = 0                                  # overflow linear cursor
            for p in range(128):
                n = int(sizes[segs[p]])
                keep = min(n, t)
                idx[p, off:off + keep] = starts[segs[p]] + np.arange(keep)
                if n > t:
                    r = starts[segs[p]] + np.arange(t, n)
                    li = l + np.arange(n - t)
                    ovx[li % 128, ovoff + li // 128] = r
                    sov[li % 128, ovoff + li // 128, p] = 1.0
                    l += n - t
            assert l <= NOV[s] * 256
            off += t
            ovoff += 2 * NOV[s]
        xdr = xq_pad[idx.reshape(-1)].reshape(128, TOT, D)
        xov = xq_pad[ovx.reshape(-1)].reshape(128, OVT, D)
        crow = sizes[segs_c].astype(np.float32).reshape(1, SEG)
        m = dict(shared)
        m.update(xdr=xdr, xov=xov, sov=sov.astype(NPF8),
                 cbc=np.repeat(crow, 128, axis=0))
        per_core.append(m)
        core_segs.append(segs_c)
    return TP, NOV, per_core, core_segs


def kernel(**inputs):
    TP, NOV, per_core, core_segs = prepare_inputs(inputs)
    nc = build_program(TP, NOV)
    res = run_bass_kernel_spmd(nc, per_core, list(range(N_CORES)))

    out = np.empty((NSEG, D), np.float32)
    for c in range(N_CORES):
        out[core_segs[c]] = res.results[c]["outT"].T
    return out


# revision 75
# speedup vs baseline: 1.0837x; 1.0521x over previous
"""Trainium2 Bass kernel for segment_reduce MLP (nn_HeadSemantic_35983236006251).

Math shortcut: Linear commutes with segment_sum, so
    pooled = segment_sum(x @ W_in + b_in) = segment_sum(x) @ W_in + counts * b_in
and the kernel reduces to memory-bound streaming of x into per-segment sums,
followed by a tiny MLP on [4096, 256].

Design (vs. the one-hot baseline):
  * x is streamed in fp8 (e4m3) instead of fp32 -- 4x less HBM traffic.
    Host-side quantization uses error feedback (sigma-delta) along each
    (segment, column) chain, so the device's exact-fp32 PSUM accumulation sees
    a segment-sum error of ~1 quantization step instead of ~sqrt(n) steps.
    Measured end-to-end rel err ~4e-3 (gate is 2e-2).
  * No per-tile one-hot build at all: segments are sorted by size on the host
    and assigned one-per-partition; x is re-laid-out in DRAM as per-partition
    row streams.  The segment-sum is then a PSUM accumulation with a CONSTANT
    doubled-identity lhsT in fp8 DoubleRow mode (2 tiles of 128 rows per
    matmul).
  * x is DMAed in big per-partition-contiguous slabs (16 KB/partition),
    spread over THREE DGE queues (SP / Activation / Pool, greedy-balanced)
    so the descriptor rings never throttle the HBM stream; Pool's queue
    front carries the small constants, SP/Act the overflow tensors.
  * Oversized segments are capped at a per-slot threshold; their excess rows
    go through a few host-built one-hot DoubleRow matmuls appended to each
    window's accumulation group (padding overhead ~6% -> ~1%).
  * The MLP runs per 64-segment chunk as soon as its window's pooled sums
    flush, overlapped with streaming of later windows; weights/activations in
    bf16 (PSUM accumulation fp32); biases fused into the DVE PSUM->SBUF
    copies (per-partition tensor_scalar add / add+relu), the counts*b_in term
    via scalar_tensor_tensor against a host-broadcast counts plane.  PSUM
    scratch alternates by chunk parity so chunk q+1's matmuls overlap chunk
    q's drain copies; only the final chunk's drain is tail latency.

Sharding: segments sorted by size desc; window w (of 32) = segments
[128w, 128w+128); slot-group s = windows [8s, 8s+8); core c takes window
8s + c of each group.  All cores share one SPMD program whose per-slot tile
counts are the group maxima.
"""

import sys
import numpy as np
import ml_dtypes
from contextlib import ExitStack

sys.path.insert(0, "/opt/trn_rl_repo")

import concourse.bass as bass
from concourse import mybir
from concourse.bass_utils import run_bass_kernel_spmd

N = 1_000_000
D = 256
NSEG = 4096
N_CORES = 8
NSLOT = 4                  # windows per core
SEG = NSLOT * 128          # segments per core
NCHUNK = 2 * NSLOT         # 64-segment MLP chunks
F32 = mybir.dt.float32
BF16 = mybir.dt.bfloat16
F8 = mybir.dt.float8e4
NPF8 = ml_dtypes.float8_e4m3
NPBF = ml_dtypes.bfloat16
SLAB_PAIRS = 32            # row-tile pairs per DMA slab (64 tiles, 16KB/part)
NQ = 3                     # x DMA queues (SP, Act, Pool)
QSLOTS = [[0, 1, 2], [3, 4, 5], [6, 7, 8]]        # queue-exclusive xbuf slots
QCAP = [99, 99, 99]
NRING = 9
DR = mybir.MatmulPerfMode.DoubleRow
ADD = mybir.AluOpType.add
MAX = mybir.AluOpType.max
MULT = mybir.AluOpType.mult
NCONST = 12                # MLP const DMAs on s_c


def _slab_plan(TP):
    """TP = tiles per slot (even).  Each slab is
    (slot, dram_tile0, npairs, first_of_slot)."""
    slabs = []
    cumslabs = []
    base = 0
    for s, tp in enumerate(TP):
        pairs = tp // 2
        k = 0
        while k < pairs:
            np_ = min(SLAB_PAIRS, pairs - k)
            slabs.append((s, base + 2 * k, np_, k == 0))
            k += np_
        cumslabs.append(len(slabs))
        base += tp
    return slabs, cumslabs


def build_program(TP, NOV):
    nc = bass.Bass()
    TOT = sum(TP)
    OVT = 2 * sum(NOV)
    PAIRS = [tp // 2 for tp in TP]
    slabs, cumslabs = _slab_plan(TP)
    NSLAB = len(slabs)

    xdr_in = nc.declare_dram_parameter("xdr", [128, TOT, D], F8, False)
    xov_in = nc.declare_dram_parameter("xov", [128, OVT, D], F8, False)
    sov_in = nc.declare_dram_parameter("sov", [128, OVT, 128], F8, False)
    id2_in = nc.declare_dram_parameter("id2", [128, 2, 128], F8, False)
    idf_in = nc.declare_dram_parameter("idf", [128, 128], F32, False)
    win_in = nc.declare_dram_parameter("win", [D, D], BF16, False)
    w1_in = nc.declare_dram_parameter("w1", [D, 2 * D], BF16, False)
    w2_in = nc.declare_dram_parameter("w2", [2 * D, D], BF16, False)
    binT_in = nc.declare_dram_parameter("binT", [128, 2], F32, False)
    b1T_in = nc.declare_dram_parameter("b1T", [128, 4], F32, False)
    b2T_in = nc.declare_dram_parameter("b2T", [128, 2], F32, False)
    cbc_in = nc.declare_dram_parameter("cbc", [128, SEG], F32, False)
    outT_ext = nc.declare_dram_parameter("outT", [D, SEG], F32, True)

    with ExitStack() as es:
        def sem(name):
            return es.enter_context(nc.semaphore(name))

        def sb(name, shape, dt):
            return es.enter_context(nc.sbuf_tensor(name, shape, dt))

        def psum(name, shape, dt):
            return es.enter_context(nc.psum_tensor(name, shape, dt))

        s_cc, s_cf, s_pe, s_fl = sem("cc"), sem("cf"), sem("pe"), sem("fl")
        s_ca, s_cb, s_cd, s_ce = sem("ca"), sem("cb"), sem("cd"), sem("ce")
        s_tr, s_ptc, s_z, s_zc = sem("tr"), sem("ptc"), sem("z"), sem("zc")
        s_h, s_hc, s_o, s_oc = sem("h"), sem("hc"), sem("o"), sem("oc")
        s_do, s_do2 = sem("do"), sem("do2")
        s_cg, s_ch, s_ov = sem("cg"), sem("ch"), sem("ov")
        s_x = [sem(f"x{i}") for i in range(NRING)]

        id2_sb = sb("id2_sb", [128, 2, 128], F8)
        xov_sb = sb("xov_sb", [128, OVT, D], F8)
        sov_sb = sb("sov_sb", [128, OVT, 128], F8)
        idf_sb = sb("idf_sb", [128, 128], F32)
        xbuf = [sb(f"xb{i}", [128, 2 * SLAB_PAIRS, D], F8) for i in range(NRING)]
        winkb = [sb(f"wink{k}", [128, D], BF16) for k in range(2)]
        w1kb = [sb(f"w1k{k}", [128, 2 * D], BF16) for k in range(2)]
        w2kb = [sb(f"w2k{k}", [128, D], BF16) for k in range(4)]
        binT = sb("binT_sb", [128, 2], F32)
        b1T = sb("b1T_sb", [128, 4], F32)
        b2T = sb("b2T_sb", [128, 2], F32)
        cbc = sb("cbc_sb", [128, SEG], F32)
        po = [sb(f"po{w}", [128, D], F32) for w in range(NSLOT)]
        pT = [sb(f"pT{k}", [128, SEG], BF16) for k in range(2)]
        zT = [sb(f"zT{k}", [128, SEG], BF16) for k in range(2)]
        hT = [sb(f"hT{j}", [128, SEG], BF16) for j in range(4)]
        ot = [sb(f"ot{j}", [128, SEG], F32) for j in range(2)]

        # 7 PSUM banks.  A[h]: per chunk parity -- cols 0:128 transposes,
        # 128:256 z, 256:384 o.  hB[h]: h-stage.  pb: stream accumulator
        # ring of 3, so a slot's first matmul never waits on the previous
        # slot's flush.
        pb = [psum(f"pb{i}", [128, D], F32) for i in range(3)]
        A = [psum("A0", [128, 512], F32), psum("A1", [128, 512], F32)]
        A2 = psum("A2", [128, 512], F32)   # fresh scratch for chunk 6 (tail)
        hB = [psum("hB0", [128, D], F32), psum("hB1", [128, D], F32)]

        def A_of(q):
            return A2 if q == 6 else A[q % 2]

        # x slab -> queue: greedy by estimated queue finish time, so slabs
        # arrive roughly in consumption order despite Pool's const preamble.
        # Ring slots are queue-exclusive (SWDGE sems must be Pool-private).
        # SP carries id2+xov up front, Act carries sov, Pool the small consts
        qload = [700.0 + OVT * 256 * 0.386,
                 700.0 + OVT * 128 * 0.386,
                 6800.0]
        qn = [0] * NQ
        queue_of = []
        for (s, t0, np_, first) in slabs:
            qi = min((i for i in range(NQ) if qn[i] < QCAP[i]),
                     key=lambda i: qload[i])
            queue_of.append(qi)
            qn[qi] += 1
            qload[qi] += np_ * 512 * 0.386 + 120
        del qload
        slot_of = [0] * NSLAB
        use_of = [0] * NSLAB      # how many times this slot was used before
        prev_user = [0] * NSLAB   # global index of the slot's previous user
        _count = {}
        _last = {}
        for g, qi in enumerate(queue_of):
            k = _count.get(qi, 0)
            pool_ = QSLOTS[qi]
            slot = pool_[k % len(pool_)]
            slot_of[g] = slot
            use_of[g] = k // len(pool_)
            prev_user[g] = _last.get(slot, -1)
            _last[slot] = g
            _count[qi] = k + 1

        def stream_queue(eng, qi):
            for g in range(NSLAB):
                if queue_of[g] != qi:
                    continue
                s, t0, np_, first = slabs[g]
                if use_of[g]:
                    eng.wait_ge(s_x[slot_of[g]], 16 * use_of[g])
                    eng.wait_ge(s_pe, prev_user[g] + 1)
                eng.dma_start(out=xbuf[slot_of[g]][:, 0:2 * np_, :],
                              in_=xdr_in[:, t0:t0 + 2 * np_, :]
                              ).then_inc(s_x[slot_of[g]], 16)

        with nc.Block(no_gpsimd_drain=True) as block:

            def out_dmas(eng, j, dsem):
                # per-chunk outputs for feature half j, chained on dsem
                for q in range(NCHUNK):
                    w, h = divmod(q, 2)
                    wch = slice(128 * w + 64 * h, 128 * w + 64 * h + 64)
                    eng.wait_ge(s_oc, 2 * q + j + 1)
                    if q:
                        eng.wait_ge(dsem, 16 * q)
                    eng.dma_start(out=outT_ext[j * 128:(j + 1) * 128, wch],
                                  in_=ot[j][:, wch]).then_inc(dsem, 16)
                eng.wait_ge(dsem, 16 * NCHUNK)

            @block.sync
            def _(sp):
                sp.dma_start(out=id2_sb[:, :, :], in_=id2_in[:, :, :]
                             ).then_inc(s_cc, 16)
                sp.dma_start(out=xov_sb[:, :, :], in_=xov_in[:, :, :]
                             ).then_inc(s_cg, 16)
                stream_queue(sp, 0)
                out_dmas(sp, 0, s_do)

            @block.scalar
            def _(a):
                a.dma_start(out=sov_sb[:, :, :], in_=sov_in[:, :, :]
                            ).then_inc(s_ch, 16)
                stream_queue(a, 1)
                out_dmas(a, 1, s_do2)

            @block.gpsimd
            def _(gp):
                # small constants first: independent same-sem chains,
                # interleaved so each link's wait is satisfied on arrival.
                chains = {
                    s_ca: [(winkb[k][:, :], win_in[k * 128:(k + 1) * 128, :])
                           for k in range(2)],
                    s_cb: [(w1kb[k][:, :], w1_in[k * 128:(k + 1) * 128, :])
                           for k in range(2)],
                    s_cd: [(w2kb[k][:, :], w2_in[k * 128:(k + 1) * 128, :])
                           for k in range(4)],
                    s_ce: [(binT[:, :], binT_in[:, :]), (b1T[:, :], b1T_in[:, :]),
                           (b2T[:, :], b2T_in[:, :]), (cbc[:, :], cbc_in[:, :])],
                    s_cf: [(idf_sb[:, :], idf_in[:, :])],
                }
                depth = {}
                for rnd in range(4):
                    for cs, lst in chains.items():
                        if rnd < len(lst):
                            k = depth.get(cs, 0)
                            if k:
                                gp.wait_ge(cs, 16 * k)
                            dst, src = lst[rnd]
                            gp.dma_start(out=dst, in_=src).then_inc(cs, 16)
                            depth[cs] = k + 1
                stream_queue(gp, 2)

            # ---- PE-side MLP stages for chunk (w, h): 64 segment columns.
            # Stages are emitted one slab apart so every wait on a DVE drain
            # is satisfied before PE reaches it (no streaming stalls).
            def _chunk(w, h):
                q = 2 * w + h
                return (q, q % 2,
                        slice(128 * w + 64 * h, 128 * w + 64 * h + 64))

            def mlp_tr(pe, w, h):
                q, cp, wch = _chunk(w, h)
                hsl = slice(64 * h, 64 * h + 64)
                if q == 0:
                    pe.wait_ge(s_cf, 16)   # idf loaded
                if q >= 2 and q != 6:
                    # chunk-parity PSUM reuse: chunk q-2's drains must be done
                    pe.wait_ge(s_ptc, 2 * (q - 1))
                    pe.wait_ge(s_zc, 2 * (q - 1))
                    pe.wait_ge(s_oc, 2 * (q - 1))
                for k in range(2):
                    pe.wait_ge(s_fl, 2 * w + k + 1)
                    pe.transpose(A_of(q)[:, k * 64:(k + 1) * 64],
                                 po[w][hsl, k * 128:(k + 1) * 128],
                                 idf_sb[hsl, hsl]).then_inc(s_tr, 1)

            def mlp_z(pe, w, h):
                # z = pooled @ W_in  (counts*b_in fused into DVE drain)
                q, cp, wch = _chunk(w, h)
                pe.wait_ge(s_ptc, 2 * q + 2)
                if q == 0:
                    pe.wait_ge(s_ca, 32)
                for j in range(2):
                    jc = slice(j * 128, (j + 1) * 128)
                    dst = A_of(q)[:, 128 + j * 64:128 + (j + 1) * 64]
                    pe.matmul(dst, winkb[0][:, jc], pT[0][:, wch], start=True, stop=False)
                    pe.matmul(dst, winkb[1][:, jc], pT[1][:, wch],
                              start=False, stop=True).then_inc(s_z, 1)

            def mlp_h(pe, w, h):
                # h = relu(z @ W1 + b1)  (bias+relu fused into DVE drain)
                q, cp, wch = _chunk(w, h)
                pe.wait_ge(s_zc, 2 * q + 2)
                if q == 0:
                    pe.wait_ge(s_cb, 32)
                if q >= 2:
                    pe.wait_ge(s_hc, 4 * (q - 1))
                for j in range(4):
                    jc = slice(j * 128, (j + 1) * 128)
                    dst = hB[cp][:, j * 64:(j + 1) * 64]
                    pe.matmul(dst, w1kb[0][:, jc], zT[0][:, wch], start=True, stop=False)
                    pe.matmul(dst, w1kb[1][:, jc], zT[1][:, wch],
                              start=False, stop=True).then_inc(s_h, 1)

            def mlp_o(pe, w, h):
                # o = h @ W2  (b2 fused into DVE drain)
                q, cp, wch = _chunk(w, h)
                pe.wait_ge(s_hc, 4 * q + 4)
                if q == 0:
                    pe.wait_ge(s_cd, 64)
                for j in range(2):
                    jc = slice(j * 128, (j + 1) * 128)
                    dst = A_of(q)[:, 256 + j * 64:256 + (j + 1) * 64]
                    for i in range(4):
                        mm = pe.matmul(dst, w2kb[i][:, jc], hT[i][:, wch],
                                       start=(i == 0), stop=(i == 3))
                    mm.then_inc(s_o, 1)

            @block.tensor
            def _(pe):
                pe.wait_ge(s_cc, 16)   # id2 loaded
                # clock warm-up: the PE p-state ramps to full speed only after
                # 3us of continuous execution; burn idle pre-stream time on
                # dummy matmuls so slab 0 is processed at full rate (and PE
                # carries a small lag buffer so arrivals stay ahead of it).
                for _ in range(WARMUP):
                    pe.matmul(A2[:, 384:512], id2_sb[:, :, :], id2_sb[:, :, :],
                              start=True, stop=True, perf_mode=DR)
                # stage schedule: window s-1's MLP stages spread over the
                # first 4 slabs of slot s, one stage (both chunks) per slab
                stage_after = {}
                for g, (s, t0, np_, first) in enumerate(slabs):
                    if first and s >= 1:
                        last = cumslabs[s] - 1
                        p = s - 1
                        for d, fn in enumerate((mlp_tr, mlp_z, mlp_h, mlp_o)):
                            stage_after.setdefault(min(g + d, last), []).extend(
                                [(fn, p, 0), (fn, p, 1)])
                # predicted arrival vs PE-ready times: fill holes with
                # dummy matmuls on A2's dead columns so the PE p-state never
                # drops back to mid rate mid-stream
                arr = [0.0] * NSLAB
                qt = [700.0 + OVT * 256 * 0.386 + 1300,
                      700.0 + OVT * 128 * 0.386 + 1300, 6800.0 + 1300]
                for g2, (s2, _t, np2, _f) in enumerate(slabs):
                    qt[queue_of[g2]] += np2 * 512 * 0.386 + 120
                    arr[g2] = qt[queue_of[g2]] + 1000
                ndum = [0] * NSLAB
                ready = arr[0]
                for g2, (s2, _t, np2, _f) in enumerate(slabs):
                    if g2:
                        gap = arr[g2] - ready
                        if gap > 60:
                            ndum[g2] = min(int(gap / 53) + 1, 60)
                    ready = max(ready + ndum[g2] * 53, arr[g2]) + np2 * 54 + 650
                    if g2 == cumslabs[s2] - 1:
                        ready += NOV[s2] * 53
                for g, (s, t0, np_, first) in enumerate(slabs):
                    for _ in range(ndum[g]):
                        pe.matmul(A2[:, 384:512], id2_sb[:, :, :],
                                  id2_sb[:, :, :], start=True, stop=True,
                                  perf_mode=DR)
                    pe.wait_ge(s_x[slot_of[g]], 16 * (use_of[g] + 1))
                    if first and s >= 3:
                        pe.wait_ge(s_fl, 2 * (s - 2))
                    k0 = (t0 - sum(TP[:s])) // 2
                    for i in range(np_):
                        kk = k0 + i
                        mm = pe.matmul(pb[s % 3][:, 0:D], id2_sb[:, :, :],
                                       xbuf[slot_of[g]][:, 2 * i:2 * i + 2, :],
                                       start=(kk == 0), stop=False,
                                       perf_mode=DR)
                        if i == np_ - 1:
                            mm.then_inc(s_pe, 1)
                    if g == cumslabs[s] - 1:
                        # overflow one-hot pairs close the accumulation group
                        if s == 0:
                            pe.wait_ge(s_cg, 16)
                            pe.wait_ge(s_ch, 16)
                        ob = 2 * sum(NOV[:s])
                        for k in range(NOV[s]):
                            mm = pe.matmul(
                                pb[s % 3][:, 0:D],
                                sov_sb[:, ob + 2 * k:ob + 2 * k + 2, :],
                                xov_sb[:, ob + 2 * k:ob + 2 * k + 2, :],
                                start=False, stop=(k == NOV[s] - 1),
                                perf_mode=DR)
                        mm.then_inc(s_ov, 1)
                    for (fn, w, h) in stage_after.get(g, []):
                        fn(pe, w, h)
                # tail: window 3 zippered
                p = NSLOT - 1
                for fn in (mlp_tr, mlp_z, mlp_h, mlp_o):
                    fn(pe, p, 0)
                    fn(pe, p, 1)

            # ---- DVE-side drains, stage granular
            def dve_ptc(v, w, h):
                q, cp, wch = _chunk(w, h)
                v.wait_ge(s_tr, 2 * q + 2)
                for k in range(2):
                    v.tensor_copy(pT[k][:, wch],
                                  A_of(q)[:, k * 64:(k + 1) * 64]).then_inc(s_ptc, 1)

            def dve_zc(v, w, h):
                q, cp, wch = _chunk(w, h)
                if q == 0:
                    v.wait_ge(s_ce, 64)   # binT/b1T/b2T/cbc loaded
                v.wait_ge(s_z, 2 * q + 2)
                for j in range(2):
                    # zT = zP + b_in[j-block] (x) counts
                    v.scalar_tensor_tensor(
                        zT[j][:, wch], cbc[:, wch], binT[:, j:j + 1],
                        A_of(q)[:, 128 + j * 64:128 + (j + 1) * 64],
                        MULT, ADD).then_inc(s_zc, 1)

            def dve_hc(v, w, h):
                q, cp, wch = _chunk(w, h)
                v.wait_ge(s_h, 4 * q + 4)
                for j in range(4):
                    # hT = relu(hP + b1[j-block])
                    v.tensor_scalar(hT[j][:, wch],
                                    hB[cp][:, j * 64:(j + 1) * 64],
                                    b1T[:, j:j + 1], 0.0, ADD, MAX).then_inc(s_hc, 1)

            def dve_oc(v, w, h):
                q, cp, wch = _chunk(w, h)
                v.wait_ge(s_o, 2 * q + 2)
                for j in range(2):
                    # ot = oP + b2[j-block]
                    v.tensor_scalar(ot[j][:, wch],
                                    A_of(q)[:, 256 + j * 64:256 + (j + 1) * 64],
                                    b2T[:, j:j + 1], None, ADD).then_inc(s_oc, 1)

            @block.vector
            def _(v):
                # drains for window w-1 that gate PE's pre-flush stages MUST
                # precede flush(w); hc(w-1,1)/oc(w-1,*) only gate stages PE
                # reaches after flush(w), so flush slots in between (it waits
                # only on s_pe, which PE raises before those stages).
                def flush(w):
                    v.wait_ge(s_pe, cumslabs[w])
                    v.wait_ge(s_ov, w + 1)
                    for k in range(2):
                        v.tensor_copy(po[w][:, k * 128:(k + 1) * 128],
                                      pb[w % 3][:, k * 128:(k + 1) * 128]
                                      ).then_inc(s_fl, 1)
                flush(0)
                for w in range(1, NSLOT + 1):
                    p = w - 1
                    dve_ptc(v, p, 0)
                    dve_ptc(v, p, 1)
                    dve_zc(v, p, 0)
                    dve_zc(v, p, 1)
                    dve_hc(v, p, 0)
                    dve_hc(v, p, 1)
                    if w < NSLOT:
                        flush(w)
                    dve_oc(v, p, 0)
                    dve_oc(v, p, 1)

    return nc


def _quantize_feedback(x, sizes, starts, order):
    """fp8 e4m3 with per-(segment, column) sigma-delta error feedback."""
    xq = np.empty(x.shape, dtype=NPF8)
    # process segments in descending-size order so live set is a prefix
    sz_d = sizes[order]                       # descending
    st_d = starts[order]
    carry = np.zeros((NSEG, D), np.float32)
    maxlen = int(sz_d[0])
    for r in range(maxlen):
        m = int(np.searchsorted(-sz_d, -(r + 1), side="right"))
        rows = st_d[:m] + r
        acc = x[rows] + carry[:m]
        q = acc.astype(NPF8)
        xq[rows] = q
        carry[:m] = acc - q.astype(np.float32)
    return xq


def _plan(batch):
    sizes = np.bincount(batch, minlength=NSEG).astype(np.int64)
    starts = np.concatenate([[0], np.cumsum(sizes)])[:-1]
    order = np.argsort(-sizes, kind="stable")
    # per slot-group: cap the stream at t_s rows/segment and push the excess
    # of oversized segments into one-hot overflow pairs; choose t_s to
    # minimize per-partition stream bytes (256/row regular, 768/ov-pair-row
    # since overflow also carries its one-hot lhsT)
    TP, NOV = [], []
    for s in range(NSLOT):
        grp = sizes[order[1024 * s:1024 * (s + 1)]]            # descending
        tmax = int(grp[0] + 1) // 2 * 2
        best = (None, None)
        for t in range(int(grp.min()) // 2 * 2, tmax + 2, 2):
            ov_core = [
                int(np.maximum(grp[c::8] - t, 0).sum()) for c in range(N_CORES)
            ]
            nov = max(1, -(-max(ov_core) // 256))
            cost = t * 256 + nov * 2 * (256 + 128)
            if best[0] is None or cost < best[0]:
                best = (cost, (t, nov))
        TP.append(best[1][0])
        NOV.append(best[1][1])
    return sizes, starts, order, TP, NOV


def prepare_inputs(inputs):
    """Host-side shard plan: returns (TP, per_core input maps, core_segs)."""
    x = np.ascontiguousarray(np.asarray(inputs["x"], np.float32))
    batch = np.asarray(inputs["batch"]).astype(np.int64)
    W_in = np.asarray(inputs["W_in"], np.float32)
    b_in = np.asarray(inputs["b_in"], np.float32).reshape(1, D)
    W1 = np.asarray(inputs["W1"], np.float32)
    b1 = np.asarray(inputs["b1"], np.float32).reshape(1, 2 * D)
    W2 = np.asarray(inputs["W2"], np.float32)
    b2 = np.asarray(inputs["b2"], np.float32).reshape(1, D)

    sizes, starts, order, TP, NOV = _plan(batch)
    TOT = sum(TP)
    OVT = 2 * sum(NOV)
    xq = _quantize_feedback(x, sizes, starts, order)
    xq_pad = np.concatenate([xq, np.zeros((1, D), NPF8)])

    id2 = np.stack([np.eye(128, dtype=np.float32)] * 2, axis=1).astype(NPF8)
    idf = np.eye(128, dtype=np.float32)
    shared = dict(
        id2=id2, idf=idf,
        win=W_in.astype(NPBF), w1=W1.astype(NPBF), w2=W2.astype(NPBF),
        binT=np.ascontiguousarray(b_in.reshape(2, 128).T),
        b1T=np.ascontiguousarray(b1.reshape(4, 128).T),
        b2T=np.ascontiguousarray(b2.reshape(2, 128).T),
    )

    per_core = []
    core_segs = []
    for c in range(N_CORES):
        idx = np.full((128, TOT), N, np.int64)
        ovx = np.full((128, OVT), N, np.int64)    # overflow row gather plan
        sov = np.zeros((128, OVT, 128), np.float32)
        segs_c = np.empty(SEG, np.int64)
        off = 0
        ovoff = 0
        for s in range(NSLOT):
            segs = order[1024 * s + c:1024 * (s + 1):8]   # strided: balanced
            segs_c[128 * s:128 * s + 128] = segs
            t = TP[s]
            l = 0                                  # overflow linear cursor
            for p in range(128):
                n = int(sizes[segs[p]])
                keep = min(n, t)
                idx[p, off:off + keep] = starts[segs[p]] + np.arange(keep)
                if n > t:
                    r = starts[segs[p]] + np.arange(t, n)
                    li = l + np.arange(n - t)
                    ovx[li % 128, ovoff + li // 128] = r
                    sov[li % 128, ovoff + li // 128, p] = 1.0
                    l += n - t
            assert l <= NOV[s] * 256
            off += t
            ovoff += 2 * NOV[s]
        xdr = xq_pad[idx.reshape(-1)].reshape(128, TOT, D)
        xov = xq_pad[ovx.reshape(-1)].reshape(128, OVT, D)
        crow = sizes[segs_c].astype(np.float32).reshape(1, SEG)
        m = dict(shared)
        m.update(xdr=xdr, xov=xov, sov=sov.astype(NPF8),
                 cbc=np.repeat(crow, 128, axis=0))
        per_core.append(m)
        core_segs.append(segs_c)
    return TP, NOV, per_core, core_segs


def kernel(**inputs):
    TP, NOV, per_core, core_segs = prepare_inputs(inputs)
    nc = build_program(TP, NOV)
    res = run_bass_kernel_spmd(nc, per_core, list(range(N_CORES)))

    out = np.empty((NSEG, D), np.float32)
    for c in range(N_CORES):
        out[core_segs[c]] = res.results[c]["outT"].T
    return out
